# revision 48
# baseline (speedup 1.0000x reference)
"""Multi-head attention layer (B=2, L=2048, D=1024, H=16) on 8 trn2 cores.

Sharding: core c handles batch b=c//4 and head group g=c%4 (4 heads, 256 dims).
Each core computes Q/K/V projections for its head-group columns, attention for
its 4 heads, and a partial out-projection (its 256 rows of Wo). The host sums
the 4 partials per batch and adds the bias terms.

Device layout notes:
- Inputs are shipped pre-transposed ([D, L]) and pre-cast to bf16 so every
  matmul contracts over the partition dim without on-device transposes.
- Q/K biases are folded in by augmenting the contraction dim with a ones row
  (row 1024) carrying bq/bk; V bias and bo commute through softmax and are
  added on the host as bv @ Wo + bo.
- Softmax skips the max-subtraction: scores*scale ~ N(0,1), |max| < ~6, safely
  inside fp32/bf16 exp range.
- Scores are computed transposed (S^T: keys on partitions, queries free) so
  exp(S^T) tiles feed the P@V matmul directly as lhsT. The row-sum for the
  softmax denominator comes from a ones column appended to V (M=65), and the
  reciprocal is broadcast across partitions with a K=1 outer-product matmul.
"""
import sys

if "/opt/trn_rl_repo" not in sys.path:
    sys.path.insert(0, "/opt/trn_rl_repo")

import numpy as np
import ml_dtypes

D_MODEL = 1024
N_HEADS = 16
HEAD_DIM = 64
B, L = 2, 2048
N_CORES = 8
GROUPS = 4                      # head groups (tensor-parallel dim)
DG = D_MODEL // GROUPS          # 256 dims per head group
HPG = N_HEADS // GROUPS         # 4 heads per group
DAUG = D_MODEL + 128            # contraction padded with bias row (9*128)
KOQ = DAUG // 128               # 9
KOV = D_MODEL // 128            # 8
KC = L // 128                   # 16 key chunks
QC = 4                          # query chunks of 512
QW = L // QC                    # 512
PAIRS = HPG // 2                # head pairs per core


def _build_program(phases=3, repeat=1, daug=DAUG):
    import os as _os
    NO_FILL = bool(int(_os.environ.get("K_NOFILL", "0")))
    NO_OPFILL = bool(int(_os.environ.get("K_NOOPFILL", "0")))
    LAG_ENV = int(_os.environ.get("K_LAG", "4"))
    import concourse.mybir as mybir
    import concourse.tile as tile
    from concourse import bacc

    fp32 = mybir.dt.float32
    bf16 = mybir.dt.bfloat16
    Exp = mybir.ActivationFunctionType.Exp
    KOQ_ = daug // 128

    nc = bacc.Bacc(None, target_bir_lowering=False)

    qT_d = nc.declare_dram_parameter("qT", [daug, L], bf16, isOutput=False)
    kT_d = nc.declare_dram_parameter("kT", [daug, L], bf16, isOutput=False)
    vT_d = nc.declare_dram_parameter("vT", [D_MODEL, L], bf16, isOutput=False)
    wq_d = nc.declare_dram_parameter("wq", [daug, DG], bf16, isOutput=False)
    wk_d = nc.declare_dram_parameter("wk", [daug, DG], bf16, isOutput=False)
    wv_d = nc.declare_dram_parameter("wv", [D_MODEL, DG], bf16, isOutput=False)
    wo_d = nc.declare_dram_parameter("wo", [DG, D_MODEL], bf16, isOutput=False)
    out_d = nc.declare_dram_parameter("out", [L, D_MODEL], fp32, isOutput=True)

    # preamble const: ones row for the K=1 broadcast matmul
    ones_t = nc.alloc_sbuf_tensor("ones_row", [128, 64], bf16)
    nc.gpsimd.memset(ones_t.ap(), 1.0)
    nc.all_engine_barrier()
    ones_ap = ones_t.ap()

    scale = 1.0 / np.sqrt(HEAD_DIM)
    LAG = LAG_ENV  # PV trails S^T/exp by this many key chunks

    with tile.TileContext(nc) as tc:
      for _rep in range(repeat):
        with (
            tc.tile_pool(name="persist", bufs=1) as pp,
            tc.tile_pool(name="staging", bufs=1) as sp,
            tc.tile_pool(name="pt", bufs=1) as ptp,
            tc.tile_pool(name="psum", bufs=1, space="PSUM") as pspool,
            tc.tile_pool(name="rsb", bufs=1) as rsb,
            tc.tile_pool(name="out_sb", bufs=4) as out_sb_p,
        ):
            wq_sb = pp.tile([128, KOQ_, DG], bf16)
            wk_sb = pp.tile([128, KOQ_, DG], bf16)
            wv_sb = pp.tile([128, KOV, DG], bf16)
            wo_sb = pp.tile([128, 2, D_MODEL], bf16)
            nc.sync.dma_start(wk_sb[:], wk_d.rearrange("(ko p) n -> p ko n", p=128))
            nc.sync.dma_start(wq_sb[:], wq_d.rearrange("(ko p) n -> p ko n", p=128))
            nc.gpsimd.dma_start(wv_sb[:], wv_d.rearrange("(ko p) n -> p ko n", p=128))

            QT = pp.tile([128, 2, L], bf16)     # [pair-dim, pair, q]
            KT = pp.tile([128, 2, L], bf16)
            Vaug = pp.tile([128, KC, HPG * 65], bf16)   # per head: [V_h | ones]
            OT = pp.tile([128, 2, L], bf16)     # attention out^T per pair

            qT_sb = sp.tile([128, KOQ_, L], bf16)
            kT_sb = sp.tile([128, KOQ_, L], bf16)
            vT_sb = sp.tile([128, KOV, L], bf16)
            qT_r = qT_d.rearrange("(ko p) f -> p ko f", p=128)
            kT_r = kT_d.rearrange("(ko p) f -> p ko f", p=128)
            vT_r = vT_d.rearrange("(ko p) f -> p ko f", p=128)

            # input DMAs in consumption order: the S^T stream (kT, then the
            # pair-0 qT column) is what keeps ACT fed, so it gets priority;
            # vT (needed by the deferred PV) follows; remaining qT last.
            for ko in range(KOQ_):
                nc.sync.dma_start(kT_sb[:, ko, 0:QW], kT_r[:, ko, 0:QW])
            for ko in range(KOQ_):
                nc.gpsimd.dma_start(qT_sb[:, ko, 0:QW], qT_r[:, ko, 0:QW])
            for ko in range(KOV):
                nc.gpsimd.dma_start(vT_sb[:, ko, 0:QW], vT_r[:, ko, 0:QW])
            for nch in range(1, QC):
                csl = slice(nch * QW, (nch + 1) * QW)
                for ko in range(KOQ_):
                    nc.sync.dma_start(kT_sb[:, ko, csl], kT_r[:, ko, csl])
            for nch in range(1, QC):
                csl = slice(nch * QW, (nch + 1) * QW)
                for ko in range(KOV):
                    nc.sync.dma_start(vT_sb[:, ko, csl], vT_r[:, ko, csl])
            for nch in range(1, QC):
                csl = slice(nch * QW, (nch + 1) * QW)
                for ko in range(KOQ_):
                    nc.gpsimd.dma_start(qT_sb[:, ko, csl], qT_r[:, ko, csl])
            nc.sync.dma_start(wo_sb[:], wo_d.rearrange("(ds p) n -> p ds n", p=128))

            for h in range(HPG):
                nc.vector.memset(Vaug[:, :, 65 * h + 64 : 65 * h + 65], 1.0)

            # ---- block emitters ----
            def v_block(kc):
                ps_t = pspool.tile([128, QW], fp32, tag="porp", bufs=2, name="vps")
                for ko in range(KOV):
                    nc.tensor.matmul(
                        ps_t[:, :DG],
                        vT_sb[:, ko, kc * 128 : (kc + 1) * 128],
                        wv_sb[:, ko, :],
                        start=(ko == 0),
                        stop=(ko == KOV - 1),
                        skip_group_check=True,
                    )
                nc.vector.tensor_copy(
                    Vaug[:, kc, :].rearrange("p (h c) -> p h c", c=65)[:, :, 0:64],
                    ps_t[:, :DG].rearrange("p (h c) -> p h c", c=64),
                )

            def proj_block(w_sb, in_sb, dst, ms, nch):
                ps_t = pspool.tile([128, QW], fp32, tag="porp", bufs=2, name="pps")
                for ko in range(KOQ_):
                    nc.tensor.matmul(
                        ps_t[:],
                        w_sb[:, ko, ms * 128 : (ms + 1) * 128],
                        in_sb[:, ko, nch * QW : (nch + 1) * QW],
                        start=(ko == 0),
                        stop=(ko == KOQ_ - 1),
                        skip_group_check=True,
                    )
                nc.vector.tensor_copy(dst[:, ms, nch * QW : (nch + 1) * QW], ps_t[:])

            def outproj_unit(qt, nch):
                oqsl = slice(qt * 128, (qt + 1) * 128)
                nsl = slice(nch * QW, (nch + 1) * QW)
                po = pspool.tile([128, QW], fp32, tag="porp", bufs=2, name="ops")
                for ds_ in range(2):
                    nc.tensor.matmul(
                        po[:],
                        OT[:, ds_, oqsl],
                        wo_sb[:, ds_, nsl],
                        start=(ds_ == 0),
                        stop=(ds_ == 1),
                        skip_group_check=True,
                    )
                ob = out_sb_p.tile([128, QW], fp32, name="ob")
                nc.vector.tensor_copy(ob[:], po[:])
                nc.sync.dma_start(out_d[oqsl, nsl], ob[:])

            # filler queue in readiness/deadline order; out-projection units
            # are appended as their OT ranges complete
            fillers = []
            for n in range(1, QC):
                fillers.append(lambda nch=n: proj_block(wk_sb, kT_sb, KT, 0, nch))
            for kc in range(KC):
                fillers.append(lambda kc=kc: v_block(kc))
            fillers.append(lambda: proj_block(wq_sb, qT_sb, QT, 0, 1))
            for n in range(2, QC):
                fillers.append(lambda nch=n: proj_block(wq_sb, qT_sb, QT, 0, nch))
            for n in range(QC):
                fillers.append(lambda nch=n: proj_block(wk_sb, kT_sb, KT, 1, nch))
            for n in range(QC):
                fillers.append(lambda nch=n: proj_block(wq_sb, qT_sb, QT, 1, nch))
            filler_pos = [0]

            def next_filler():
                if filler_pos[0] < len(fillers):
                    f = fillers[filler_pos[0]]
                    filler_pos[0] += 1
                    return f
                return None

            def emit_fillers(n):
                if NO_FILL:
                    return
                for _ in range(n):
                    f = next_filler()
                    if f is not None:
                        f()

            # ---- lead-in: pair-0 first column blocks (V comes via fillers) ----
            proj_block(wk_sb, kT_sb, KT, 0, 0)
            proj_block(wq_sb, qT_sb, QT, 0, 0)
            if NO_FILL:
                for f in fillers:
                    f()

            # ---- attention with interleaved fillers ----
            pending_norm = [None]

            def emit_pending_norm():
                if pending_norm[0] is not None:
                    pending_norm[0]()
                    pending_norm[0] = None

            for pair in range(PAIRS):
                for qc in range(QC):
                    qsl = slice(qc * QW, (qc + 1) * QW)
                    PT0s, PT1s = {}, {}
                    pv0 = pspool.tile([128, QW], fp32, tag="pv0", bufs=1, name="pv0")
                    pv1 = pspool.tile([128, QW], fp32, tag="pv1", bufs=1, name="pv1")

                    def st_step(kc):
                        ksl = slice(kc * 128, (kc + 1) * 128)
                        # both heads' score chunks land in one 2-bank psum
                        # tile so a single exp covers the pair
                        s01 = pspool.tile([128, 2 * QW], fp32, tag="s01",
                                          bufs=2, name="s01")
                        nc.tensor.matmul(
                            s01[:, 0:QW], KT[0:64, pair, ksl],
                            QT[0:64, pair, qsl],
                            start=True, stop=True, skip_group_check=True,
                        )
                        nc.tensor.matmul(
                            s01[:, QW : 2 * QW], KT[64:128, pair, ksl],
                            QT[64:128, pair, qsl],
                            start=True, stop=True, skip_group_check=True,
                        )
                        pt = ptp.tile([128, 2 * QW], bf16, tag="PT", bufs=11,
                                      name="pt")
                        nc.scalar.activation(pt[:], s01[:], Exp, scale=scale)
                        PT0s[kc], PT1s[kc] = pt[:, 0:QW], pt[:, QW : 2 * QW]

                    def pv_step(kc):
                        h0 = 2 * pair
                        nc.tensor.matmul(
                            pv0[0:65, :],
                            Vaug[:, kc, 65 * h0 : 65 * h0 + 65],
                            PT0s.pop(kc),
                            start=(kc == 0), stop=(kc == KC - 1),
                            skip_group_check=True,
                        )
                        nc.tensor.matmul(
                            pv1[0:65, :],
                            Vaug[:, kc, 65 * (h0 + 1) : 65 * (h0 + 1) + 65],
                            PT1s.pop(kc),
                            start=(kc == 0), stop=(kc == KC - 1),
                            skip_group_check=True,
                        )

                    first = pair == 0 and qc == 0
                    last = pair == PAIRS - 1 and qc == QC - 1
                    lag = 8 if first else (2 if last else LAG)
                    if first:
                        slot_plan = {1: 1, 3: 1, 5: 2, 7: 2, 9: 2, 11: 2,
                                     13: 2, 15: 2}
                        post = 6
                    elif pair == 0:
                        slot_plan = {3: 1, 7: 1, 11: 1, 15: 1}
                        post = 0
                    else:
                        slot_plan = {3: 2, 7: 2, 11: 2, 15: 2}
                        post = 0
                    for kc in range(KC):
                        st_step(kc)
                        if kc == 1:
                            emit_pending_norm()
                        if kc in slot_plan:
                            emit_fillers(slot_plan[kc])
                        if kc >= lag:
                            pv_step(kc - lag)
                    emit_fillers(post)
                    for kc in range(KC - lag, KC):
                        pv_step(kc)

                    # softmax denominator (deferred into the next qc's S^T
                    # stream): recip -> K=1 broadcast -> mul
                    def norm(pair=pair, qsl=qsl, pv0=pv0, pv1=pv1):
                        for h01, pv in ((0, pv0), (1, pv1)):
                            rc = rsb.tile([128, QW], fp32, tag="rc", bufs=2,
                                          name="rc")
                            nc.vector.reciprocal(rc[0:1, :], pv[64:65, :])
                            rcb = rsb.tile([128, QW], bf16, tag="rcb", bufs=2,
                                           name="rcb")
                            nc.vector.tensor_copy(rcb[0:1, :], rc[0:1, :])
                            rp = pspool.tile([64, QW], fp32, tag="porp", bufs=2,
                                             name="rp")
                            nc.tensor.matmul(
                                rp[:], ones_ap[0:1, :], rcb[0:1, :],
                                start=True, stop=True, skip_group_check=True,
                            )
                            rpsb = rsb.tile([128, QW], fp32, tag="rpsb", bufs=2,
                                            name="rpsb")
                            nc.vector.tensor_copy(rpsb[0:64, :], rp[:])
                            nc.vector.tensor_tensor(
                                OT[64 * h01 : 64 * h01 + 64, pair, qsl],
                                pv[0:64, :],
                                rpsb[0:64, :],
                                mybir.AluOpType.mult,
                            )

                    pending_norm[0] = norm
                    if pair == 1 and not NO_OPFILL:
                        for qi in range(QW // 128):
                            qt = qc * (QW // 128) + qi
                            for nch in range(2):
                                fillers.append(
                                    lambda qt=qt, nch=nch: outproj_unit(qt, nch))

            emit_pending_norm()
            # drain remaining fillers (final out-projection chunks)
            while True:
                f = next_filler()
                if f is None:
                    break
                f()
            if NO_OPFILL:
                for qt in range(L // 128):
                    for nch in range(2):
                        outproj_unit(qt, nch)

    nc.compile()
    return nc


class _Runner:
    """Persistent PJRT executable: build/trace once, execute many times.

    Mirrors bass2jax.run_bass_via_pjrt's multi-core shard_map path, but keeps
    the jitted callable (and the NEFF) alive across calls and skips output
    donation — this kernel writes every output element, so pre-zeroed output
    buffers aren't needed.
    """

    def __init__(self, build_fn=None):
        import jax
        import numpy as _np
        from jax.sharding import Mesh, PartitionSpec
        from jax.experimental.shard_map import shard_map
        import concourse.mybir as mybir
        from concourse import bass2jax

        bass2jax.install_neuronx_cc_hook()
        self.nc = nc = (build_fn or _build_program)()
        self.jax = jax

        partition_name = (
            nc.partition_id_tensor.name if nc.partition_id_tensor else None
        )
        in_names, out_names, out_avals = [], [], []
        for alloc in nc.m.functions[0].allocations:
            if not isinstance(alloc, mybir.MemoryLocationSet):
                continue
            name = alloc.memorylocations[0].name
            if alloc.kind == "ExternalInput":
                if name != partition_name:
                    in_names.append(name)
            elif alloc.kind == "ExternalOutput":
                out_names.append(name)
                out_avals.append(
                    jax.core.ShapedArray(
                        tuple(alloc.tensor_shape), mybir.dt.np(alloc.dtype)
                    )
                )
        self.in_names, self.out_names, self.out_avals = in_names, out_names, out_avals
        n_params = len(in_names)
        zero_outs = [
            _np.zeros((N_CORES * a.shape[0], *a.shape[1:]), a.dtype) for a in out_avals
        ]

        body_in_names = in_names + out_names
        if partition_name is not None:
            body_in_names = body_in_names + [partition_name]

        def _body(*args):
            operands = list(args)
            if partition_name is not None:
                operands.append(bass2jax.partition_id_tensor())
            outs = bass2jax._bass_exec_p.bind(
                *operands,
                out_avals=tuple(out_avals),
                in_names=tuple(body_in_names),
                out_names=tuple(out_names),
                lowering_input_output_aliases=(),
                sim_require_finite=True,
                sim_require_nnan=True,
                nc=nc,
            )
            return tuple(outs)

        self._body = _body
        devices = jax.devices()[:N_CORES]
        self.mesh = Mesh(_np.asarray(devices), ("core",))
        in_specs = (PartitionSpec("core"),) * (n_params + len(out_names))
        out_specs = (PartitionSpec("core"),) * len(out_names)
        self.fn = jax.jit(
            shard_map(
                _body,
                mesh=self.mesh,
                in_specs=in_specs,
                out_specs=out_specs,
                check_rep=False,
            ),
            keep_unused=True,
        )
        self.sharding = jax.sharding.NamedSharding(self.mesh, PartitionSpec("core"))
        self.zeros_dev = [jax.device_put(z, self.sharding) for z in zero_outs]

    def make_chained(self, k):
        """Jitted fn executing the NEFF k times back-to-back (output buffers
        threaded into the next call), for marginal-cost timing."""
        import numpy as _np
        from jax.sharding import PartitionSpec
        from jax.experimental.shard_map import shard_map

        n_params = len(self.in_names)
        body = self._body

        def _chain(*args):
            ins, outs = args[:n_params], args[n_params:]
            for _ in range(k):
                outs = body(*ins, *outs)
            return outs

        in_specs = (PartitionSpec("core"),) * (n_params + len(self.out_names))
        out_specs = (PartitionSpec("core"),) * len(self.out_names)
        return self.jax.jit(
            shard_map(
                _chain,
                mesh=self.mesh,
                in_specs=in_specs,
                out_specs=out_specs,
                check_rep=False,
            ),
            keep_unused=True,
        )

    def put_inputs(self, in_maps):
        import numpy as _np

        concat = [
            _np.concatenate([m[name] for m in in_maps], axis=0)
            for name in self.in_names
        ]
        return [self.jax.device_put(c, self.sharding) for c in concat]

    def execute(self, dev_inputs):
        outs = self.fn(*dev_inputs, *self.zeros_dev)
        self.jax.block_until_ready(outs)
        return outs

    def run(self, in_maps):
        import numpy as _np

        outs = self.execute(self.put_inputs(in_maps))
        return [
            {
                name: _np.asarray(outs[i]).reshape(
                    N_CORES, *self.out_avals[i].shape
                )[c]
                for i, name in enumerate(self.out_names)
            }
            for c in range(N_CORES)
        ]


_RUNNERS = {}


def _get_runner(daug=DAUG):
    if daug not in _RUNNERS:
        _RUNNERS[daug] = _Runner(lambda: _build_program(daug=daug))
    return _RUNNERS[daug]


def kernel(query, key, value, Wq, bq, Wk, bk, Wv, bv, Wo, bo, **extra):

    bf = ml_dtypes.bfloat16
    query = np.asarray(query, np.float32)
    key = np.asarray(key, np.float32)
    value = np.asarray(value, np.float32)
    Wq = np.asarray(Wq, np.float32)
    Wk = np.asarray(Wk, np.float32)
    Wv = np.asarray(Wv, np.float32)
    Wo = np.asarray(Wo, np.float32)
    bq = np.asarray(bq, np.float32)
    bk = np.asarray(bk, np.float32)
    bv = np.asarray(bv, np.float32)
    bo = np.asarray(bo, np.float32)

    # zero q/k biases (the generated case) skip the bias-augmented row
    daug = D_MODEL if (not bq.any() and not bk.any()) else DAUG
    runner = _get_runner(daug)

    # host-side shard prep: transpose + bias-augment + cast
    def aug_T(x):  # [L, D] -> [daug, L] (ones row at 1024 when augmented)
        if daug == D_MODEL:
            return np.ascontiguousarray(x.T).astype(bf)
        xa = np.zeros((daug, L), np.float32)
        xa[:D_MODEL] = x.T
        xa[D_MODEL] = 1.0
        return xa.astype(bf)

    def aug_W(w, b):  # [D, DG-slice] (+ bias row when augmented)
        if daug == D_MODEL:
            return np.ascontiguousarray(w).astype(bf)
        wa = np.zeros((daug, w.shape[1]), np.float32)
        wa[:D_MODEL] = w
        wa[D_MODEL] = b
        return wa.astype(bf)

    qTs = [aug_T(query[b]) for b in range(B)]
    kTs = [aug_T(key[b]) for b in range(B)]
    vTs = [value[b].T.astype(bf) for b in range(B)]

    in_maps = []
    for c in range(N_CORES):
        b, g = divmod(c, GROUPS)
        gs = slice(g * DG, (g + 1) * DG)
        in_maps.append({
            "qT": qTs[b],
            "kT": kTs[b],
            "vT": vTs[b],
            "wq": aug_W(Wq[:, gs], bq[gs]),
            "wk": aug_W(Wk[:, gs], bk[gs]),
            "wv": Wv[:, gs].astype(bf),
            "wo": Wo[gs, :].astype(bf),
        })

    global _LAST_IN_MAPS
    _LAST_IN_MAPS = in_maps
    results = runner.run(in_maps)

    host_bias = (bv.astype(np.float32) @ Wo.astype(np.float32)) + bo
    out = np.zeros((B, L, D_MODEL), np.float32)
    for c in range(N_CORES):
        b = c // GROUPS
        out[b] += results[c]["out"]
    out += host_bias
    return out
